# revision 3
# baseline (speedup 1.0000x reference)
"""Chebyshev graph-conv kernel for Trainium2 (8 NeuronCores, SPMD).

Math: out[b,o,m,t] = sum_{k,c,n} T[k,n,m] * x[b,c,n,t] * Theta[k,c,o]
with T the Chebyshev polynomials of the normalized adjacency (n=24, K=3).

The whole operator collapses into a single 768x768 matrix
    W[(c,n),(o,m)] = sum_k Theta[k,c,o] * T[k,n,m]
applied per batch element to x[b] viewed as (c*n, t) = (768, 512):
    out[b](o*24+m, t) = W.T-contract over rows -> exactly one matmul chain.

W is tiny and computed on host from adj/Theta; x is read once and out
written once. Data-parallel over batch: 64 -> 8 per core. x, W and the
output all move as fp16 (PE multiplies 16-bit operands at full rate with
hidden weight loads; fp16 I/O halves HBM traffic in both directions; the
host upcasts the result to fp32). PSUM accumulation is full fp32; the
PSUM->SBUF copy does the fp32->fp16 cast. Per core: 8 batch elements,
each a 6x6 chain of [128,128]x[128,512] matmuls accumulated in PSUM.

Schedule: batch 0 runs chunk-outer across 6 concurrent PSUM chains so
matmuls start as soon as chunk 0 of W and x0 arrives (instead of waiting
for the full 2 MB); W/x0 preamble loads are interleaved chunk-by-chunk on
two HWDGE rings. Batches 1-7 run chain-outer so chains retire (and store)
staggered. With fp16 stores total DMA (14.6 MB) sits well under the PE
time (~62 us), so stores just flow on the scalar ring unconstrained.
"""

import numpy as np

import concourse.mybir as mybir
from concourse import bacc, tile
from concourse.bass_utils import run_bass_kernel_spmd

N_CORES = 8
B, C, NV, T = 64, 32, 24, 512
K = 3
O = 32
CN = C * NV   # 768 contraction rows
OM = O * NV   # 768 output rows
BP = B // N_CORES  # 8 batch elements per core
P = 128
NBLK = CN // P  # 6

_compiled_nc = None
last_result = None  # BassKernelResults from the most recent run (for test.py)


def _build_nc():
    f32 = mybir.dt.float32
    f16 = mybir.dt.float16
    nc = bacc.Bacc("TRN2", target_bir_lowering=False, debug=False,
                   num_devices=N_CORES)
    xs = nc.dram_tensor("xs", [BP, CN, T], f16, kind="ExternalInput")
    w = nc.dram_tensor("w", [CN, OM], f16, kind="ExternalInput")
    out = nc.dram_tensor("out", [BP, OM, T], f16, kind="ExternalOutput")

    wr = w[:].rearrange("(i p) m -> p i m", p=P)

    with tile.TileContext(nc) as tc:
        with (
            tc.tile_pool(name="wpool", bufs=1) as wpool,
            tc.tile_pool(name="xpool", bufs=BP) as xpool,
            tc.tile_pool(name="opool", bufs=6) as opool,
            tc.tile_pool(name="psum", bufs=8, space="PSUM") as psum_pool,
        ):
            # HAM warm-up: the PE boots throttled at K=4/8 (1.2 GHz) and
            # unthrottles after ~3.4us of sustained busy. Dummy matmuls on a
            # zeroed tile cover the data-starved preamble (~2.5us until the
            # first W/x chunk pair lands); the first real matmuls finish the
            # warm-up budget.
            warm = wpool.tile([P, T], f16, tag="warm")
            nc.gpsimd.memset(warm[:], 0.0)
            for _ in range(6):
                wps = psum_pool.tile([P, T], f32, tag="ps")
                nc.tensor.matmul(wps[:], warm[:, :P], warm[:], start=True, stop=True)

            # Preamble: interleave W and x0 chunk loads on two HWDGE rings
            # (x on Sync, W on Scalar -- stores only start ~7us in, so the
            # Scalar ring is free) so chunk pair i lands every ~0.9us and the
            # batch-0 chunk-outer matmuls below can start on pair 0.
            wt = wpool.tile([P, NBLK, OM], f16)
            xt0 = xpool.tile([P, NBLK, T], f16, tag="xt0")
            xr0 = xs[0].rearrange("(i p) t -> p i t", p=P)
            for i in range(NBLK):
                nc.scalar.dma_start(wt[:, i, :], wr[:, i, :])
                nc.sync.dma_start(xt0[:, i, :], xr0[:, i, :])

            xts = [xt0]
            for b in range(1, BP):
                xt = xpool.tile([P, NBLK, T], f16, tag="xt0")
                xr = xs[b].rearrange("(i p) t -> p i t", p=P)
                nc.sync.dma_start(xt[:], xr)
                xts.append(xt)

            # Batch 0: chunk-outer across 6 concurrent PSUM chains. Each
            # chunk pair (w_i, x0_i) enables one matmul per chain, so the PE
            # ramps with the loads instead of idling until everything lands.
            ps0 = [psum_pool.tile([P, T], f32, tag="ps", name=f"ps0_{j}")
                   for j in range(NBLK)]
            for i in range(NBLK):
                for j in range(NBLK):
                    nc.tensor.matmul(
                        ps0[j][:],
                        wt[:, i, j * P:(j + 1) * P],
                        xt0[:, i, :],
                        start=(i == 0),
                        stop=(i == NBLK - 1),
                    )
            ot0 = opool.tile([P, NBLK, T], f16)
            orr0 = out[0].rearrange("(j p) t -> p j t", p=P)
            for j in range(NBLK):
                nc.vector.tensor_copy(ot0[:, j, :], ps0[j][:])
                nc.scalar.dma_start(orr0[:, j, :], ot0[:, j, :])

            # Batches 1-7: chain-outer; chains retire staggered so the
            # fp32->fp16 copies and stores spread across the whole window.
            for b in range(1, BP):
                xt = xts[b]
                ot = opool.tile([P, NBLK, T], f16)
                orr = out[b].rearrange("(j p) t -> p j t", p=P)
                for j in range(NBLK):
                    ps = psum_pool.tile([P, T], f32)
                    for i in range(NBLK):
                        nc.tensor.matmul(
                            ps[:],
                            wt[:, i, j * P:(j + 1) * P],
                            xt[:, i, :],
                            start=(i == 0),
                            stop=(i == NBLK - 1),
                        )
                    nc.vector.tensor_copy(ot[:, j, :], ps[:])
                    nc.scalar.dma_start(orr[:, j, :], ot[:, j, :])

    nc.compile()
    return nc


def _combined_operator(adj: np.ndarray, Theta: np.ndarray) -> np.ndarray:
    """W[(c,n),(o,m)] = sum_k Theta[k,c,o] * T[k,n,m], shape (768,768) fp16."""
    adj = np.asarray(adj).astype(np.float32)
    Theta = np.asarray(Theta)
    d = adj.sum(axis=1)
    d_inv_sqrt = np.where(d > 0, 1.0 / np.sqrt(d), 0.0).astype(np.float32)
    L = (adj * d_inv_sqrt[None, :]).T * d_inv_sqrt[None, :]
    Ts = [np.eye(NV, dtype=np.float32), L.astype(np.float32)]
    for _ in range(2, K):
        Ts.append((2.0 * L @ Ts[-1] - Ts[-2]).astype(np.float32))
    Tcheb = np.stack(Ts[:K])  # (K, n, m)
    W = np.einsum("kco,knm->cnom", Theta.astype(np.float32), Tcheb)
    return np.ascontiguousarray(W.reshape(CN, OM), dtype=np.float16)


def kernel(x: np.ndarray, adj: np.ndarray, Theta: np.ndarray) -> np.ndarray:
    global _compiled_nc, last_result
    if _compiled_nc is None:
        _compiled_nc = _build_nc()
    nc = _compiled_nc

    W = _combined_operator(adj, Theta)
    # x: (64, 32, 24, 512) -> per-core shard [8, 768, 512], fp16 (the device
    # matmul consumes fp16 regardless; casting host-side halves HBM reads)
    xf = np.asarray(x).astype(np.float16).reshape(B, CN, T)
    in_maps = [
        {"xs": np.ascontiguousarray(xf[c * BP:(c + 1) * BP]), "w": W}
        for c in range(N_CORES)
    ]
    res = run_bass_kernel_spmd(nc, in_maps, core_ids=list(range(N_CORES)))
    last_result = res
    out = np.concatenate([r["out"] for r in res.results], axis=0)
    return np.ascontiguousarray(
        out.reshape(B, O, NV, T).astype(np.float32))
